# revision 1
# baseline (speedup 1.0000x reference)
"""Trainium2 Bass kernel: ViT-style global attention with decomposed
relative position bias (B=8, 32x32 tokens, dim 768, 12 heads, hd 64).

Sharding: data-parallel over batch B=8 -> one image per NeuronCore (8
cores), weights replicated, no collectives. TimelineSim: ~206.3 us/core.

Per-core dataflow (all on-chip ops partition-preserving; fp32r matmuls
at full PE rate, bf16 only where precision allows):
  1a  q/k features = Wqk @ xT (feature-major). q pre-scaled by hd^-0.5
      via host weight fold; per-partition bias applied during the
      PSUM->SBUF drain (ACT for q -> Q'ALL halves, DVE for k -> K'ALL
      halves + a bf16 staging copy STQB for the rel-pos matmuls).
      Odd heads are row-mirrored (rows 64:127) so every copy preserves
      partition indices.
  1b  RELH/RELW = rel-pos tables contracted against q: per 4 token-rows,
      one bf16 matmul with a block-diagonal stationary (prefetched,
      host-packed) computes even+odd heads of all 6 head-pairs at once;
      drained into Q'ALL = [qT | RELH | RELW] per head.
  1c  V token-major (xT stationary), V-bias via K=1 ones-row matmul,
      stored bf16 with a ones column and parity-dependent layout
      [V|1] / [0,1,0,V] -> softmax denominators and the AOD partition
      placement ride the PV matmul for free.
  2   attention per (head, kblock): ONE K=128 fp32r matmul produces
      scale*S^T + rel_h + rel_w in PSUM (the bias rides contraction rows
      64..127 against constant 0/1 indicator patterns stored in K'ALL).
      exp on ScalarE (PSUM->SBUF, bf16) -> P^T. PV matmul with the V''
      stationary accumulates (P@V)^T + the denominator row over kblocks.
      A K=1 ones-matmul broadcasts the denominator row; reciprocal
      (PSUM->SBUF) + multiply on DVE -> AOD feature-major. Denominators
      skip the max-subtraction (logits are bounded ~|2.5|) - safe in
      fp32. proj weights prefetched during attention.
  3   proj matmul over AOD (+proj_b per-partition on DVE) -> y^T
      feature-major, DMA'd out; the final transpose to token-major
      happens on the host during unsharding.
"""

import numpy as np

import concourse.bacc as bacc
import concourse.tile as tile
from concourse import mybir
from concourse import bass_utils

B, H, W, DIM = 8, 32, 32, 768
HEADS, HD = 12, 64
N = H * W  # 1024
NCORES = 8
SCALE = HD ** -0.5
F32 = mybir.dt.float32
F32R = mybir.dt.float32r
BF16 = mybir.dt.bfloat16
EXP = mybir.ActivationFunctionType.Exp
ADD = mybir.AluOpType.add

NC = DIM // 128      # 6 feature chunks
NKB = N // 128       # 8 key blocks
NQH = N // 512       # 2 query halves
VW = 65 + 128        # even (V|1) + odd (0,1,0,V) stationary widths

_CACHE = {}

import os
KNOB_KCOPY_ACT = os.environ.get("K_KCOPY", "0") == "1"   # k he-half on ACT
KNOB_STQB_ACT = os.environ.get("K_STQB", "0") == "1"     # STQB copy on ACT
KNOB_WCOPY_ACT = os.environ.get("K_WCOPY", "0") == "1"   # relw copies on ACT
KNOB_HCOPY_DVE = os.environ.get("K_HCOPY", "0") == "1"   # relh copies on DVE
KNOB_PT_BUFS = int(os.environ.get("K_PT", "7"))
KNOB_VCOPY_DVE = os.environ.get("K_VCOPY", "0") == "1"   # V copies on DVE


def build_nc():
    nc = bacc.Bacc("TRN2", target_bir_lowering=False, debug=False)

    xT = nc.dram_tensor("xT", (DIM, N), F32R, kind="ExternalInput").ap()
    wqkvT = nc.dram_tensor("wqkvT", (DIM, 3 * DIM), F32R, kind="ExternalInput").ap()
    qkvb = nc.dram_tensor("qkvb", (3 * DIM,), F32, kind="ExternalInput").ap()
    wprojT = nc.dram_tensor("wprojT", (DIM, DIM), F32R, kind="ExternalInput").ap()
    projb = nc.dram_tensor("projb", (DIM,), F32, kind="ExternalInput").ap()
    bdh = nc.dram_tensor("bdh", (128, H, 128), BF16, kind="ExternalInput").ap()
    bdw = nc.dram_tensor("bdw", (128, W, 128), BF16, kind="ExternalInput").ap()
    kconst = nc.dram_tensor("kconst", (64, N), F32R, kind="ExternalInput").ap()
    consd = nc.dram_tensor("consd", (128, 256), F32R, kind="ExternalInput").ap()
    vbrow = nc.dram_tensor("vbrow", (1, DIM), F32R, kind="ExternalInput").ap()
    y = nc.dram_tensor("y", (DIM, N), F32, kind="ExternalOutput").ap()

    qkvb2 = qkvb.rearrange("(c p one) -> c p one", p=128, one=1)   # [18][128,1]
    projb2 = projb.rearrange("(c p one) -> c p one", p=128, one=1)  # [6][128,1]
    bdh_r = bdh
    bdw_r = bdw
    IDN = mybir.ActivationFunctionType.Identity

    with tile.TileContext(nc) as tc:
        # PE p-state warm-up: the first real matmuls are DMA-gated for
        # ~12us while the PE would ramp at half rate for its first 3us of
        # busy time. Fill the idle window with throwaway matmuls so the
        # ramp completes before the first real matmul issues.
        nwarm = int(os.environ.get("K_WARM", "37"))
        if nwarm:
            with tc.tile_pool(name="warm", bufs=1) as warm_p, \
                 tc.tile_pool(name="warmps", bufs=1, space="PSUM") as wps_p:
                jnk = warm_p.tile([128, 512], BF16)
                nc.vector.memset(jnk, 0.5)
                jps = wps_p.tile([128, 512], F32)
                for _ in range(nwarm):
                    nc.tensor.matmul(jps, lhsT=jnk[:, 0:128], rhs=jnk,
                                     start=True, stop=True,
                                     skip_group_check=True)

        # ---- long-lived pools (bottom of SBUF stack) ----
        qall_p = tc.alloc_tile_pool(name="qall", bufs=1)
        kall_p = tc.alloc_tile_pool(name="kall", bufs=1)
        vall_p = tc.alloc_tile_pool(name="vall", bufs=1)
        cons_p = tc.alloc_tile_pool(name="cons", bufs=1)

        QALL = qall_p.tile([128, HEADS, N], F32R)
        KALL = kall_p.tile([128, HEADS, N], F32R)
        VALL = vall_p.tile([128, HEADS // 2, NKB, VW], BF16)
        CONS = cons_p.tile([128, 256], F32R)
        VBS = cons_p.tile([1, DIM], F32R)

        xt_p = tc.alloc_tile_pool(name="xtp", bufs=1)
        XT = xt_p.tile([128, NC, N], F32R, tag="xtslot")
        with tc.tile_pool(name="stage", bufs=1) as stage_p, \
             tc.tile_pool(name="wpool", bufs=1) as w_p, \
             tc.tile_pool(name="bias", bufs=4) as b_p, \
             tc.tile_pool(name="ps1", bufs=4, space="PSUM") as ps1_p:
            STQB = stage_p.tile([128, NC, N], BF16)
            # ---------- 1a: q & k features (feature-major) ----------
            def emit_qkv_group(g, first=False):
                wt = w_p.tile([128, NC, 384], F32R, tag="wt", bufs=2, name=f"wt{g}")
                for c in range(NC):
                    nc.sync.dma_start(
                        out=wt[:, c, :],
                        in_=wqkvT[c * 128:(c + 1) * 128, g * 384:(g + 1) * 384])
                for mi in range(3):
                    m = g * 3 + mi       # 0..11 (q: 0-5, k: 6-11)
                    bias_t = b_p.tile([128, 1], F32, tag="bias", name=f"b{m}")
                    nc.sync.dma_start(out=bias_t, in_=qkvb2[m])
                    for qh in range(NQH):
                        ps = ps1_p.tile([128, 512], F32, tag="ps1", bufs=int(os.environ.get("K_PS1","4")),
                                        name=f"ps1_{m}_{qh}")
                        for c in range(NC):
                            nc.tensor.matmul(
                                ps,
                                lhsT=wt[:, c, mi * 128:(mi + 1) * 128],
                                rhs=XT[:, c, qh * 512:(qh + 1) * 512],
                                start=(c == 0), stop=(c == NC - 1))
                        qsl = slice(qh * 512, (qh + 1) * 512)
                        he, ho = 2 * (m % 6), 2 * (m % 6) + 1
                        if m < 6:
                            nc.scalar.activation(QALL[0:64, he, qsl], ps[0:64],
                                                 IDN, bias=bias_t[0:64],
                                                 scale=1.0)
                            nc.scalar.activation(QALL[64:128, ho, qsl],
                                                 ps[64:128], IDN,
                                                 bias=bias_t[64:128], scale=1.0)
                            if KNOB_STQB_ACT:
                                nc.scalar.activation(STQB[:, m, qsl], ps, IDN,
                                                     bias=bias_t, scale=1.0)
                            else:
                                nc.vector.tensor_scalar(
                                    out=STQB[:, m, qsl], in0=ps,
                                    scalar1=bias_t, scalar2=None, op0=ADD)
                        else:
                            if KNOB_KCOPY_ACT:
                                nc.scalar.activation(
                                    KALL[0:64, he, qsl], ps[0:64], IDN,
                                    bias=bias_t[0:64], scale=1.0)
                            else:
                                nc.vector.tensor_scalar(
                                    out=KALL[0:64, he, qsl], in0=ps[0:64],
                                    scalar1=bias_t[0:64], scalar2=None, op0=ADD)
                            nc.vector.tensor_scalar(
                                out=KALL[64:128, ho, qsl], in0=ps[64:128],
                                scalar1=bias_t[64:128], scalar2=None, op0=ADD)

            for qh in range(NQH):
                for c in range(NC):
                    nc.sync.dma_start(
                        out=XT[:, c, qh * 512:(qh + 1) * 512],
                        in_=xT[c * 128:(c + 1) * 128, qh * 512:(qh + 1) * 512])
            emit_qkv_group(0)
            emit_qkv_group(1)
            bdts = []
            for i in range(16):
                src_r = bdh_r if i < 8 else bdw_r
                j0 = (i % 8) * 4
                bdt = w_p.tile([128, 4, 128], BF16, tag="bd", bufs=16,
                               name=f"bdt{i}")
                nc.sync.dma_start(out=bdt, in_=src_r[:, j0:j0 + 4, :])
                bdts.append(bdt)
            for g in range(2, 4):
                emit_qkv_group(g)

            # constants (after the critical 1a DMAs so they don't gate PE)
                nc.sync.dma_start(out=VBS, in_=vbrow)
            nc.sync.dma_start(out=CONS, in_=consd)
            # V'' layout: even head cols [V(64)|1]; odd [0(32)|1|0(31)|V(64)]
            nc.vector.memset(VALL[:, :, :, 64:65], 1.0)
            nc.vector.memset(VALL[:, :, :, 65:97], 0.0)
            nc.vector.memset(VALL[:, :, :, 97:98], 1.0)
            nc.vector.memset(VALL[:, :, :, 98:129], 0.0)

            # ---------- 1b: RELH/RELW ----------
            # out views with batched-h free-dim order (hb, pair, t)
            qvh = QALL.rearrange("p (pr hh) (hb t) -> p hb pr hh t", hh=2, t=W)
            qvw = QALL.rearrange("p (pr hh) (t wb) -> p wb pr hh t", hh=2, wb=W)
            stq4 = STQB.rearrange("p c (t ww) -> p c t ww", ww=W)
            RG = int(os.environ.get("K_RG", "4"))
            for i0 in range(0, H, RG):
                bdh_t = bdts[i0 // 4]
                ps_h = ps1_p.tile([128, RG, 256], F32, tag="ps2",
                                  bufs=int(os.environ.get("K_PS2", "2")),
                                  name=f"psh{i0}")
                for j in range(RG):
                    h = i0 + j
                    nc.tensor.matmul(
                        ps_h[:, j, 0:192].rearrange("p (c t) -> p c t", t=32),
                        lhsT=bdts[h // 4][:, h % 4, :],
                        rhs=STQB[:, :, h * 32:(h + 1) * 32],
                        start=True, stop=True, skip_group_check=True)
                _hcp = nc.vector.tensor_copy if KNOB_HCOPY_DVE else nc.scalar.copy
                _hcp(qvh[64:96, i0:i0 + RG, :, 0, :],
                     ps_h[64:96, :, 0:192].rearrange(
                         "p hb (c t) -> p hb c t", t=32))
                _hcp(qvh[0:32, i0:i0 + RG, :, 1, :],
                     ps_h[0:32, :, 0:192].rearrange(
                         "p hb (c t) -> p hb c t", t=32))
                ps_w = ps1_p.tile([128, RG, 256], F32, tag="ps2",
                                  bufs=int(os.environ.get("K_PS2", "2")),
                                  name=f"psw{i0}")
                for j in range(RG):
                    w = i0 + j
                    nc.tensor.matmul(
                        ps_w[:, j, 0:192].rearrange("p (c t) -> p c t", t=32),
                        lhsT=bdts[8 + w // 4][:, w % 4, :],
                        rhs=stq4[:, :, :, w],
                        start=True, stop=True, skip_group_check=True)
                _wcp = nc.scalar.copy if KNOB_WCOPY_ACT else nc.vector.tensor_copy
                _wcp(qvw[96:128, i0:i0 + RG, :, 0, :],
                     ps_w[96:128, :, 0:192].rearrange(
                         "p wb (c t) -> p wb c t", t=32))
                _wcp(qvw[32:64, i0:i0 + RG, :, 1, :],
                     ps_w[32:64, :, 0:192].rearrange(
                         "p wb (c t) -> p wb c t", t=32))

            # ---------- 1c: V token-major (vh=0: heads 0-5) ----------
            def emit_v_half(vh, wt, pool, tag, bufs):
                for tb in range(NKB):    # 8 token blocks
                    ps = pool.tile([128, 512], F32, tag=tag,
                                   bufs=int(os.environ.get("K_PS1", "4")),
                                   name=f"psv{vh}_{tb}")[:, 0:384]
                    for c in range(NC):
                        nc.tensor.matmul(
                            ps, lhsT=XT[:, c, tb * 128:(tb + 1) * 128],
                            rhs=wt[:, c, :],
                            start=(c == 0), stop=False)
                    nc.tensor.matmul(
                        ps, lhsT=CONS[0:1, 0:128],
                        rhs=VBS[:, vh * 384:(vh + 1) * 384],
                        start=False, stop=True)
                    psj = ps.rearrange("p (jh par h) -> p jh par h", par=2, h=64)
                    import concourse.bass as bass
                    vsrc = VALL[:, vh * 3:(vh + 1) * 3, tb, 0:64]
                    vdst = bass.AP(tensor=vsrc.tensor, offset=vsrc.offset,
                                   ap=[list(vsrc.ap[0]),
                                       [VW * NKB, 3], [129, 2], [1, 64]])
                    nc.scalar.copy(vdst, psj)

            wtv0 = w_p.tile([128, NC, 384], F32R, tag="wt", bufs=2)
            for c in range(NC):
                nc.sync.dma_start(
                    out=wtv0[:, c, :],
                    in_=wqkvT[c * 128:(c + 1) * 128, 2 * DIM:2 * DIM + 384])
            emit_v_half(0, wtv0, ps1_p, "ps1", 4)
            wtv1 = w_p.tile([128, NC, 384], F32R, tag="wt", bufs=2)
            for c in range(NC):
                nc.sync.dma_start(
                    out=wtv1[:, c, :],
                    in_=wqkvT[c * 128:(c + 1) * 128, 2 * DIM + 384:3 * DIM])
            emit_v_half(1, wtv1, ps1_p, "ps1", 4)

            for h in range(HEADS):
                rows = slice(64, 128) if h % 2 == 0 else slice(0, 64)
                nc.sync.dma_start(out=KALL[rows, h, :], in_=kconst)

        # ---------- 2: attention ----------
        aod_p = tc.alloc_tile_pool(name="aod", bufs=1)
        AOD = aod_p.tile([128, NC, N], F32R)
        w2a_p = tc.alloc_tile_pool(name="w2a", bufs=1)
        WP0 = w2a_p.tile([128, NC, 384], F32R)
        PBIAS = w2a_p.tile([128, NC], F32)
        for c in range(NC):
            nc.sync.dma_start(
                out=WP0[:, c, :], in_=wprojT[c * 128:(c + 1) * 128, 0:384])
        nc.sync.dma_start(
            out=PBIAS, in_=projb.rearrange("(c p) -> p c", p=128))
        with tc.tile_pool(name="pt", bufs=KNOB_PT_BUFS) as pt_p, \
             tc.tile_pool(name="sm", bufs=int(os.environ.get("K_SM","4"))) as sm_p, \
             tc.tile_pool(name="pss", bufs=int(os.environ.get("K_PSS","2")), space="PSUM") as psS_p, \
             tc.tile_pool(name="pspv", bufs=4, space="PSUM") as psPV_p:
            for head in range(HEADS):
                pair, par = head // 2, head % 2
                pv = [psPV_p.tile([128, 512], F32, tag="pv", bufs=int(os.environ.get("K_PV","4")), name=f"pv{head}_{qh}")
                      for qh in range(NQH)]
                vsl = (slice(0, 65) if par == 0 else slice(65, 193))
                for kb in range(NKB):
                    ps_s = psS_p.tile([128, 1024], F32, tag="pss")
                    for qh in range(NQH):
                        nc.tensor.matmul(
                            ps_s[:, qh * 512:(qh + 1) * 512],
                            lhsT=KALL[:, head, kb * 128:(kb + 1) * 128],
                            rhs=QALL[:, head, qh * 512:(qh + 1) * 512],
                            start=True, stop=True)
                    pt = pt_p.tile([128, 1024], BF16, tag="pt")
                    nc.scalar.activation(pt, ps_s, EXP)
                    for qh in range(NQH):
                        pv_out = pv[qh][0:65] if par == 0 else pv[qh]
                        nc.tensor.matmul(
                            pv_out, lhsT=VALL[:, pair, kb, vsl],
                            rhs=pt[:, qh * 512:(qh + 1) * 512],
                            start=(kb == 0), stop=(kb == NKB - 1))
                dr = 64 if par == 0 else 32     # denominator row (32-aligned)
                ao_rows = slice(0, 64) if par == 0 else slice(64, 128)
                for qh in range(NQH):
                    dsb = sm_p.tile([128, 512], F32R, tag="dsb",
                                    name=f"dsb{head}_{qh}")
                    nc.vector.tensor_copy(dsb[dr:dr + 1], pv[qh][dr:dr + 1])
                    rb = psPV_p.tile([128, 512], F32, tag="pv", bufs=int(os.environ.get("K_PV","4")),
                                     name=f"rb{head}_{qh}")
                    if par == 0:
                        nc.tensor.matmul(rb[0:64], lhsT=CONS[64:65, 0:64],
                                         rhs=dsb[64:65], start=True, stop=True)
                    else:
                        nc.tensor.matmul(rb, lhsT=CONS[32:33, 128:256],
                                         rhs=dsb[32:33], start=True, stop=True)
                    rbr = sm_p.tile([128, 512], F32, tag="rbr",
                                    name=f"rbr{head}_{qh}")
                    nc.vector.reciprocal(rbr[ao_rows], rb[ao_rows])
                    nc.vector.tensor_mul(
                        AOD[ao_rows, pair, qh * 512:(qh + 1) * 512],
                        pv[qh][ao_rows], rbr[ao_rows])

        # ---------- 3: proj + bias + transpose + out ----------
        with tc.tile_pool(name="wp", bufs=1) as w2_p, \
             tc.tile_pool(name="ps4", bufs=int(os.environ.get("K_PS4","4")), space="PSUM") as ps4_p:
            YSB = xt_p.tile([128, NC, N], F32, tag="xtslot")
            WP1 = w2_p.tile([128, NC, 384], F32R)
            for c in range(NC):
                nc.sync.dma_start(
                    out=WP1[:, c, :],
                    in_=wprojT[c * 128:(c + 1) * 128, 384:768])
            wt2s = [WP0, WP1]
            for ob in range(NC):
                for qh in range(NQH):
                    g, mi = ob // 3, ob % 3
                    ps = ps4_p.tile([128, 512], F32, tag="ps4",
                                    name=f"psp{qh}_{ob}")
                    for c in range(NC):
                        nc.tensor.matmul(
                            ps,
                            lhsT=wt2s[g][:, c, mi * 128:(mi + 1) * 128],
                            rhs=AOD[:, c, qh * 512:(qh + 1) * 512],
                            start=(c == 0), stop=(c == NC - 1))
                    nc.vector.tensor_scalar(
                        out=YSB[:, ob, qh * 512:(qh + 1) * 512], in0=ps,
                        scalar1=PBIAS[:, ob:ob + 1], scalar2=None, op0=ADD)
            for ob in range(NC):
                nc.sync.dma_start(out=y[ob * 128:(ob + 1) * 128, :],
                                  in_=YSB[:, ob, :])
        w2a_p.release()
        aod_p.release()
        xt_p.release()
        cons_p.release()
        vall_p.release()
        kall_p.release()
        qall_p.release()

    nc.compile()
    return nc


def host_prep(x, qkv_w, qkv_b, proj_w, proj_b, rel_pos_h, rel_pos_w):
    """full inputs -> list of 8 per-core in_maps"""
    x = np.asarray(x, np.float32)
    qkv_w = np.asarray(qkv_w, np.float32)
    qkv_b = np.asarray(qkv_b, np.float32)
    proj_w = np.asarray(proj_w, np.float32)
    proj_b = np.asarray(proj_b, np.float32)
    rel_pos_h = np.asarray(rel_pos_h, np.float32)
    rel_pos_w = np.asarray(rel_pos_w, np.float32)

    wqkvT = np.ascontiguousarray(qkv_w.T).copy()
    wqkvT[:, :DIM] *= SCALE
    qkvb2 = qkv_b.copy()
    qkvb2[:DIM] *= SCALE
    wprojT = np.ascontiguousarray(proj_w.T)

    idx = np.arange(H)
    Rh = rel_pos_h[idx[:, None] - idx[None, :] + (H - 1)]  # (32,32,64)
    Rw = rel_pos_w[idx[:, None] - idx[None, :] + (W - 1)]
    import ml_dtypes
    bdh = np.zeros((H, 128, 128), ml_dtypes.bfloat16)
    bdw = np.zeros((W, 128, 128), ml_dtypes.bfloat16)
    for h in range(H):
        bdh[h, 0:64, 64:96] = Rh[h].T / SCALE
        bdh[h, 64:128, 0:32] = Rh[h].T / SCALE
    for w in range(W):
        bdw[w, 0:64, 96:128] = Rw[w].T / SCALE
        bdw[w, 64:128, 32:64] = Rw[w].T / SCALE
    bdh = np.ascontiguousarray(bdh.transpose(1, 0, 2))  # [128, H, 128]
    bdw = np.ascontiguousarray(bdw.transpose(1, 0, 2))

    k = np.arange(N)
    kconst = np.zeros((64, N), np.float32)
    kconst[:32] = (k[None, :] // 32 == np.arange(32)[:, None])
    kconst[32:] = (k[None, :] % 32 == np.arange(32)[:, None])

    consd = np.zeros((128, 256), np.float32)
    consd[:, 0:128] = 1.0
    consd[:, 192:256] = 1.0
    vbrow = np.ascontiguousarray(qkvb2[2 * DIM:].reshape(1, DIM))

    shared = dict(wqkvT=wqkvT, qkvb=qkvb2, wprojT=wprojT, projb=proj_b,
                  bdh=bdh, bdw=bdw, kconst=kconst,
                  consd=consd, vbrow=vbrow)
    in_maps = []
    for b in range(B):
        xT = np.ascontiguousarray(x[b].reshape(N, DIM).T)
        in_maps.append(dict(xT=xT, **shared))
    return in_maps


def get_nc():
    if "nc" not in _CACHE:
        _CACHE["nc"] = build_nc()
    return _CACHE["nc"]


def kernel(**inputs):
    nc = get_nc()
    in_maps = host_prep(**inputs)
    res = bass_utils.run_bass_kernel_spmd(nc, in_maps, core_ids=list(range(NCORES)))
    out = np.stack([np.asarray(r["y"]).T for r in res.results], axis=0)
    return np.ascontiguousarray(out).reshape(B, H, W, DIM).astype(np.float32)



# revision 20
# speedup vs baseline: 1.2625x; 1.2625x over previous
"""Trainium2 Bass kernel: ViT-style global attention with decomposed
relative position bias (B=8, 32x32 tokens, dim 768, 12 heads, hd 64).

Sharding: data-parallel over batch B=8 -> one image per NeuronCore,
weights replicated, no collectives.

v2 design (fp8 + phase restructure), per core:
  * QKV projection in fp8e4 DoubleRow, 3-pass residual form:
    host ships x and 32*W as fp8 (hi, lo=residual) pairs; kernel computes
    x_hi@W_hi + x_lo@W_hi + x_hi@W_lo (lo@lo dropped, ~bf16 accuracy) at
    0.5 cycles/row -> 2304 cycles per 128x512 output tile vs 3072 fp32r.
    Drains scale by 1/32 and add the bias (ACT for q -> QALL8 fp8,
    DVE for k -> KALL8 fp8).
  * Attention scores in ONE fp8 DoubleRow matmul per (head, kblock, qh):
    contraction pair = (64 q-features, 64 rel-bias rows).  QALL8/KALL8
    hold per-head [64p, 2, N] slices (even heads partitions 0:64, odd
    64:128); slice1 carries 8*rel values (q side) and 0/1 indicator rows
    (k side).  exp on ACT applies the 1/8 softmax scale -> bf16 P.
  * rel-pos matmuls read q directly from QALL8 (fp8, no staging copy);
    block-diagonal bd tables are fp8 with columns placed so copies into
    QALL8 slice1 stay partition-aligned (ACT even/h, Pool odd/w).
  * V per head-pair: 9 fp8 DoubleRow matmuls per 2-token-blocks + bf16
    K=1 ones-row matmul rides the 32x v-bias; Pool drains into the
    parity-coded V'' = [V|1] / [0,1,0,V] bf16 layout (32x scaled; the
    proj drain divides by 32).  Denominator rides the PV matmul.
  * Phase order: q chunks -> rel-pos -> k chunk 0 -> attention heads
    0..11 with k chunks 1..5 and V pairs 1..5 interleaved into PE slack
    (the attention phase is ACT(exp)-bound); proj fp32r at the tail.
"""

import os
import numpy as np

import concourse.bacc as bacc
import concourse.bass as bass
import concourse.tile as tile
from concourse import mybir
from concourse import bass_utils

B, H, W, DIM = 8, 32, 32, 768
HEADS, HD = 12, 64
N = H * W  # 1024
NCORES = 8
SCALE = HD ** -0.5
WS = 32.0           # fp8 weight pre-scale
F32 = mybir.dt.float32
F32R = mybir.dt.float32r
BF16 = mybir.dt.bfloat16
F8 = mybir.dt.float8e4
EXP = mybir.ActivationFunctionType.Exp
IDN = mybir.ActivationFunctionType.Identity
ADD = mybir.AluOpType.add
MULT = mybir.AluOpType.mult
DR = mybir.MatmulPerfMode.DoubleRow

NC = DIM // 128      # 6 feature chunks
NPAIR = NC // 2      # 3 contraction chunk-pairs for DoubleRow
NKB = N // 128       # 8 key blocks
NQH = N // 512       # 2 query halves
NPR = HEADS // 2     # 6 head pairs
VW = 65 + 128        # even (V|1) + odd (0,1,0,V) stationary widths

_CACHE = {}

K_WARM = int(os.environ.get("K_WARM", "12"))
K_PT = int(os.environ.get("K_PT", "10"))
K_PS1 = int(os.environ.get("K_PS1", "4"))
K_PS2 = int(os.environ.get("K_PS2", "2"))
K_PSS = int(os.environ.get("K_PSS", "2"))
K_SM = int(os.environ.get("K_SM", "4"))
K_PS4 = int(os.environ.get("K_PS4", "4"))


def build_nc():
    nc = bacc.Bacc("TRN2", target_bir_lowering=False, debug=False)

    x8hi_d = nc.dram_tensor("x8hi", (128, NC, N), F8, kind="ExternalInput").ap()
    x8lo_d = nc.dram_tensor("x8lo", (128, NC, N), F8, kind="ExternalInput").ap()
    w8hi_d = nc.dram_tensor("w8hi", (128, NPAIR, 2, 3 * DIM), F8,
                            kind="ExternalInput").ap()
    w8lo_d = nc.dram_tensor("w8lo", (128, NPAIR, 2, 3 * DIM), F8,
                            kind="ExternalInput").ap()
    qkvb_d = nc.dram_tensor("qkvb18", (128, 18), F32, kind="ExternalInput").ap()
    bd8_d = nc.dram_tensor("bd8", (128, 2, 32, 128), F8,
                           kind="ExternalInput").ap()
    kc8_d = nc.dram_tensor("kc8rep", (128, NPR, N), F8,
                           kind="ExternalInput").ap()
    vbrow_d = nc.dram_tensor("vbrow32", (1, DIM), BF16, kind="ExternalInput").ap()
    consd_d = nc.dram_tensor("consd", (128, 256), F32R, kind="ExternalInput").ap()
    wprojT_d = nc.dram_tensor("wprojT", (DIM, DIM), F32R, kind="ExternalInput").ap()
    projb_d = nc.dram_tensor("projb6", (128, NC), F32, kind="ExternalInput").ap()
    y = nc.dram_tensor("y", (DIM, N), F32, kind="ExternalOutput").ap()

    with tile.TileContext(nc) as tc:
        # PE p-state warm-up under the initial DMA gate.
        if K_WARM:
            with tc.tile_pool(name="warm", bufs=1) as warm_p, \
                 tc.tile_pool(name="warmps", bufs=1, space="PSUM") as wps_p:
                jnk = warm_p.tile([128, 512], BF16)
                nc.vector.memset(jnk, 0.5)
                jps = wps_p.tile([128, 512], F32)
                for _ in range(K_WARM):
                    nc.tensor.matmul(jps, lhsT=jnk[:, 0:128], rhs=jnk,
                                     start=True, stop=True,
                                     skip_group_check=True)

        # ---- long-lived pools (bottom of SBUF stack) ----
        qall_p = tc.alloc_tile_pool(name="qall", bufs=1)
        kall_p = tc.alloc_tile_pool(name="kall", bufs=1)
        vall_p = tc.alloc_tile_pool(name="vall", bufs=1)
        cons_p = tc.alloc_tile_pool(name="cons", bufs=1)
        aod_p = tc.alloc_tile_pool(name="aod", bufs=1)
        w2a_p = tc.alloc_tile_pool(name="w2a", bufs=1)
        xt8_p = tc.alloc_tile_pool(name="xt8", bufs=1)
        w8_p = tc.alloc_tile_pool(name="w8p", bufs=1)

        # [par*64+r, slice, pair, token]; s0 = q/k features, s1 = bias rows
        QALL8 = qall_p.tile([128, 2, NPR, N], F8)
        KALL8 = kall_p.tile([128, 2, NPR, N], F8)
        VALL = vall_p.tile([128, NPR, NKB, VW], BF16)
        CONSR = cons_p.tile([128, 256], F32R)
        CONSB = cons_p.tile([1, 128], BF16)
        VBS = cons_p.tile([1, DIM], BF16)
        QKVB = cons_p.tile([128, 18], F32)
        AOD = aod_p.tile([128, NC, N], F32R)
        WP = w2a_p.tile([128, NC, 2, 384], F32R)   # [:, c, g, :] proj cols
        PBIAS = w2a_p.tile([128, NC], F32)
        XT8HI = xt8_p.tile([128, NPAIR, 2, N], F8)
        XT8LO = xt8_p.tile([128, NPAIR, 2, N], F8)
        W8HI = w8_p.tile([128, NPAIR, 2, 3 * DIM], F8)
        W8LO = w8_p.tile([128, NPAIR, 2, 3 * DIM], F8)

        # ---- input DMAs, issue order = criticality ----
        nc.sync.dma_start(out=XT8HI.rearrange("p pr s t -> p (pr s) t"),
                          in_=x8hi_d)
        nc.sync.dma_start(out=W8HI[:, :, :, 0:DIM], in_=w8hi_d[:, :, :, 0:DIM])
        nc.sync.dma_start(out=XT8LO.rearrange("p pr s t -> p (pr s) t"),
                          in_=x8lo_d)
        nc.sync.dma_start(out=W8LO[:, :, :, 0:DIM], in_=w8lo_d[:, :, :, 0:DIM])
        nc.sync.dma_start(out=QKVB, in_=qkvb_d)

        nc.vector.memset(CONSB, 1.0)
        # V'' constant columns
        nc.vector.memset(VALL[:, :, :, 64:65], 1.0)
        nc.vector.memset(VALL[:, :, :, 65:97], 0.0)
        nc.vector.memset(VALL[:, :, :, 97:98], 1.0)
        nc.vector.memset(VALL[:, :, :, 98:129], 0.0)

        def emit_qkv_half(m, qh, ps_pool, tag="ps1", bufs=None):
            """feature chunk m (q 0..5, k 6..11), query half qh; 3-pass DR."""
            ps = ps_pool.tile([128, 512], F32, tag=tag, bufs=bufs or K_PS1,
                              name=f"ps1_{m}_{qh}")
            qsl = slice(qh * 512, (qh + 1) * 512)
            first = True
            for wt, xt in ((W8HI, XT8HI), (W8HI, XT8LO), (W8LO, XT8HI)):
                for p in range(NPAIR):
                    nc.tensor.matmul(
                        ps, lhsT=wt[:, p, :, m * 128:(m + 1) * 128],
                        rhs=xt[:, p, :, qsl],
                        start=first, stop=(wt is W8LO and p == NPAIR - 1),
                        perf_mode=DR)
                    first = False
            bias_t = QKVB[:, m:m + 1]
            if m < 6:
                nc.scalar.activation(QALL8[:, 0, m, qsl], ps, IDN,
                                     bias=bias_t, scale=1.0 / WS)
            else:
                nc.vector.tensor_scalar(
                    out=KALL8[:, 0, m - 6, qsl], in0=ps,
                    scalar1=1.0 / WS, scalar2=bias_t, op0=MULT, op1=ADD)

        def emit_v_pair(pair, ps_pool, tb0, ntb, tag="psv", bufs=1):
            """v features for head pair; 2 token-blocks per psum group."""
            for g0 in range(tb0, tb0 + ntb, 2):
                psv = ps_pool.tile([128, 2, 128], F32, tag=tag, bufs=bufs,
                                   name=f"psv{pair}_{g0}")
                for j in range(2):
                    tb = g0 + j
                    tsl = slice(tb * 128, (tb + 1) * 128)
                    vsl = slice(2 * DIM + pair * 128, 2 * DIM + (pair + 1) * 128)
                    first = True
                    for wt, xt in ((W8HI, XT8HI), (W8HI, XT8LO), (W8LO, XT8HI)):
                        for p in range(NPAIR):
                            nc.tensor.matmul(
                                psv[:, j, :], lhsT=xt[:, p, :, tsl],
                                rhs=wt[:, p, :, vsl],
                                start=first, stop=False, perf_mode=DR)
                            first = False
                    nc.tensor.matmul(
                        psv[:, j, :], lhsT=CONSB,
                        rhs=VBS[:, pair * 128:(pair + 1) * 128],
                        start=False, stop=True)
                # drain both token blocks; parity-coded V'' destination
                vsrc = VALL[:, pair, g0, 0:64]
                vdst = bass.AP(tensor=vsrc.tensor, offset=vsrc.offset,
                               ap=[list(vsrc.ap[0]),
                                   [VW, 2], [129, 2], [1, 64]])
                psj = psv.rearrange("p tb (par h) -> p tb par h", h=64)
                nc.vector.tensor_copy(vdst, psj)

        # ---------- phase 1: q chunks, rel-pos, k chunk 0 ----------
        with tc.tile_pool(name="bd", bufs=1) as bd_p, \
             tc.tile_pool(name="ps1", bufs=1, space="PSUM") as ps1_p:
            BD = bd_p.tile([128, 2, 32, 128], F8)
            nc.sync.dma_start(out=BD, in_=bd8_d)
            # remaining weight/const DMAs (after critical path issues)
            nc.sync.dma_start(out=W8HI[:, :, :, DIM:2 * DIM],
                              in_=w8hi_d[:, :, :, DIM:2 * DIM])
            nc.sync.dma_start(out=W8LO[:, :, :, DIM:2 * DIM],
                              in_=w8lo_d[:, :, :, DIM:2 * DIM])
            nc.sync.dma_start(out=KALL8[:, 1, :, :], in_=kc8_d)
            nc.sync.dma_start(out=W8HI[:, :, :, 2 * DIM:],
                              in_=w8hi_d[:, :, :, 2 * DIM:])
            nc.sync.dma_start(out=W8LO[:, :, :, 2 * DIM:],
                              in_=w8lo_d[:, :, :, 2 * DIM:])
            nc.sync.dma_start(out=VBS, in_=vbrow_d)
            nc.sync.dma_start(out=CONSR, in_=consd_d)

            # rel-pos views: h copies are row-grouped, w copies col-grouped
            q8s0 = QALL8[:, 0]                                  # [128, 6, N]
            q8col = q8s0.rearrange("p c (t ww) -> p c t ww", ww=W)
            q8s1h = QALL8[:, 1].rearrange("p c (hb t) -> p hb c t", t=W)
            q8s1w = QALL8[:, 1].rearrange("p c (t wb) -> p wb c t", wb=W)
            RG = 4

            def emit_rel_h(i0, early):
                """h-values + explicit zeros in the w-partitions: ONE
                full-width copy per group (the bd stationary has zero
                columns outside its own partitions)."""
                ps_h = ps1_p.tile([128, RG, 256], F32, tag="ps2", bufs=K_PS2,
                                  name=f"psh{i0}")
                for j in range(RG):
                    h = i0 + j
                    nc.tensor.matmul(
                        ps_h[:, j, 0:192].rearrange("p (c t) -> p c t", t=32),
                        lhsT=BD[:, 0, h, :],
                        rhs=q8s0[:, :, h * 32:(h + 1) * 32],
                        start=True, stop=True, skip_group_check=True)
                rsl = slice(i0, i0 + RG)
                pr = ps_h[:, :, 0:192].rearrange("p hb (c t) -> p hb c t",
                                                 t=32)
                if early:
                    nc.vector.tensor_copy(q8s1h[:, rsl, :, :], pr)
                else:
                    nc.scalar.copy(q8s1h[:, rsl, :, :], pr)

            def emit_rel_w(i0, acc):
                """w-values: either a DVE full-width accumulate onto the
                h-copy zeros, or two narrow ACT copies of just the
                w-partitions (disjoint from the h rows)."""
                ps_w = ps1_p.tile([128, RG, 256], F32, tag="ps2", bufs=K_PS2,
                                  name=f"psw{i0}")
                for j in range(RG):
                    w = i0 + j
                    nc.tensor.matmul(
                        ps_w[:, j, 0:192].rearrange("p (c t) -> p c t", t=32),
                        lhsT=BD[:, 1, w, :],
                        rhs=q8col[:, :, :, w],
                        start=True, stop=True, skip_group_check=True)
                rsl = slice(i0, i0 + RG)
                pr = ps_w[:, :, 0:192].rearrange("p wb (c t) -> p wb c t",
                                                 t=32)
                if acc:
                    dst = q8s1w[:, rsl, :, :]
                    nc.vector.tensor_tensor(out=dst, in0=dst, in1=pr, op=ADD)
                else:
                    nc.scalar.copy(q8s1w[32:64, rsl, :, :], pr[32:64])
                    nc.scalar.copy(q8s1w[96:128, rsl, :, :], pr[96:128])

            # q chunks qh0; then qh1 interleaved with rel h-row groups that
            # only need qh0 tokens (rows 0..15); w matmuls need all tokens
            # and their accumulating copies must follow ALL h copies.
            for m in range(6):
                emit_qkv_half(m, 0, ps1_p)
            for m in range(6):
                emit_qkv_half(m, 1, ps1_p)
                if m >= 2:
                    emit_rel_h((m - 2) * RG, early=True)
            for i0 in range(16, H, RG):
                emit_rel_h(i0, early=False)
            # w wave, with k chunk 0 and V pair 0 in the PE slack
            emit_rel_w(0, True)
            emit_qkv_half(6, 0, ps1_p)
            emit_rel_w(4, False)
            emit_qkv_half(6, 1, ps1_p)
            emit_rel_w(8, True)
            emit_rel_w(12, False)
            emit_v_pair(0, ps1_p, 0, 2, tag='ps1', bufs=K_PS1)
            emit_rel_w(16, True)
            emit_v_pair(0, ps1_p, 2, 2, tag='ps1', bufs=K_PS1)
            emit_rel_w(20, False)
            emit_v_pair(0, ps1_p, 4, 2, tag='ps1', bufs=K_PS1)
            emit_rel_w(24, True)
            emit_v_pair(0, ps1_p, 6, 2, tag='ps1', bufs=K_PS1)
            emit_rel_w(28, True)

        # ---------- phase 2: attention (ACT-bound), fillers in PE slack ----
        with tc.tile_pool(name="pt", bufs=K_PT) as pt_p, \
             tc.tile_pool(name="sm", bufs=K_SM) as sm_p, \
             tc.tile_pool(name="pss", bufs=K_PSS, space="PSUM") as psS_p, \
             tc.tile_pool(name="pv", bufs=2, space="PSUM") as psPV_p, \
             tc.tile_pool(name="rb", bufs=1, space="PSUM") as psRB_p, \
             tc.tile_pool(name="psvp", bufs=1, space="PSUM") as psV_p:


            def filler(head, kb):
                """PE slack work, emitted inside the kb loop (kb 2/4/6) so the
                next head's S matmuls are never far behind in program order."""
                c = head // 2 + 1
                if c > 5:
                    if head == 10 and kb == 2:
                        nc.sync.dma_start(
                            out=WP[:, :, 0, :],
                            in_=wprojT_d[:, 0:384].rearrange(
                                "(c p) f -> p c f", p=128))
                        nc.sync.dma_start(
                            out=WP[:, :, 1, :],
                            in_=wprojT_d[:, 384:768].rearrange(
                                "(c p) f -> p c f", p=128))
                        nc.sync.dma_start(out=PBIAS, in_=projb_d)
                    return
                if head % 2 == 0:
                    if kb == 2:
                        emit_qkv_half(6 + c, 0, psV_p, tag="psv", bufs=1)
                    elif kb == 4:
                        emit_qkv_half(6 + c, 1, psV_p, tag="psv", bufs=1)
                    else:
                        emit_v_pair(c, psV_p, 0, 2)
                else:
                    emit_v_pair(c, psV_p, 2 * (kb // 2), 2)

            def emit_s_exp(head, kb):
                pair, par = head // 2, head % 2
                p0 = par * 64
                ps_s = psS_p.tile([128, 1024], F32, tag="pss")
                ksl = slice(kb * 128, (kb + 1) * 128)
                for qh in range(NQH):
                    nc.tensor.matmul(
                        ps_s[:, qh * 512:(qh + 1) * 512],
                        lhsT=KALL8[p0:p0 + 64, :, pair, ksl],
                        rhs=QALL8[p0:p0 + 64, :, pair,
                                  qh * 512:(qh + 1) * 512],
                        start=True, stop=True, perf_mode=DR)
                pt = pt_p.tile([128, 1024], BF16, tag="pt")
                nc.scalar.activation(pt, ps_s, EXP, scale=SCALE)
                return pt

            def emit_pv(head, kb, pv, pt):
                pair, par = head // 2, head % 2
                vsl = (slice(0, 65) if par == 0 else slice(65, 193))
                for qh in range(NQH):
                    pv_out = pv[qh][0:65] if par == 0 else pv[qh]
                    nc.tensor.matmul(
                        pv_out, lhsT=VALL[:, pair, kb, vsl],
                        rhs=pt[:, qh * 512:(qh + 1) * 512],
                        start=(kb == 0), stop=(kb == NKB - 1))

            def emit_norm(head, pv):
                pair, par = head // 2, head % 2
                dr_row = 64 if par == 0 else 32
                ao_rows = slice(0, 64) if par == 0 else slice(64, 128)
                for qh in range(NQH):
                    dsb = sm_p.tile([128, 512], F32R, tag="dsb",
                                    name=f"dsb{head}_{qh}")
                    nc.vector.tensor_copy(dsb[dr_row:dr_row + 1],
                                          pv[qh][dr_row:dr_row + 1])
                    rbt = psRB_p.tile([128, 512], F32, tag="rb",
                                      name=f"rb{head}_{qh}")
                    if par == 0:
                        nc.tensor.matmul(rbt[0:64], lhsT=CONSR[64:65, 0:64],
                                         rhs=dsb[64:65], start=True, stop=True)
                    else:
                        nc.tensor.matmul(rbt, lhsT=CONSR[32:33, 128:256],
                                         rhs=dsb[32:33], start=True, stop=True)
                    rbr = sm_p.tile([128, 512], F32, tag="rbr",
                                    name=f"rbr{head}_{qh}")
                    nc.vector.reciprocal(rbr[ao_rows], rbt[ao_rows])
                    nc.vector.tensor_mul(
                        AOD[ao_rows, pair, qh * 512:(qh + 1) * 512],
                        pv[qh][ao_rows], rbr[ao_rows])

            # software-pipelined two ways: every PV is emitted after the NEXT
            # step's S+exp (a stalled PV in the PE weight-load queue would
            # otherwise block the following S), and head h's norm is emitted
            # after head h+1's first S+exp so the exp chain never waits.
            prev_norm = None
            pending = None          # (head, kb, pv, pt) for the deferred PV
            pv = None
            for head in range(HEADS):
                for kb in range(NKB):
                    pt = emit_s_exp(head, kb)
                    if pending is not None:
                        emit_pv(*pending)
                    if kb == 0:
                        if prev_norm is not None:
                            emit_norm(prev_norm[0], prev_norm[1])
                        pv = [psPV_p.tile([128, 512], F32, tag="pv", bufs=2,
                                          name=f"pv{head}_{qh}")
                              for qh in range(NQH)]
                    pending = (head, kb, pv, pt)
                    if kb in (2, 4, 6):
                        filler(head, kb)
                prev_norm = (head, pv)
            emit_pv(*pending)
            emit_norm(prev_norm[0], prev_norm[1])

        # ---------- phase 3: proj (fp32r) + 1/32 descale + bias ----------
        with tc.tile_pool(name="ysb", bufs=1) as ysb_p, \
             tc.tile_pool(name="ps4", bufs=K_PS4, space="PSUM") as ps4_p:
            YSB = ysb_p.tile([128, NC, N], F32)
            for ob in range(NC):
                for qh in range(NQH):
                    g, mi = ob // 3, ob % 3
                    ps = ps4_p.tile([128, 512], F32, tag="ps4",
                                    name=f"psp{qh}_{ob}")
                    for c in range(NC):
                        nc.tensor.matmul(
                            ps,
                            lhsT=WP[:, c, g, mi * 128:(mi + 1) * 128],
                            rhs=AOD[:, c, qh * 512:(qh + 1) * 512],
                            start=(c == 0), stop=(c == NC - 1))
                    nc.vector.tensor_scalar(
                        out=YSB[:, ob, qh * 512:(qh + 1) * 512], in0=ps,
                        scalar1=1.0 / WS, scalar2=PBIAS[:, ob:ob + 1],
                        op0=MULT, op1=ADD)
                nc.sync.dma_start(out=y[ob * 128:(ob + 1) * 128, :],
                                  in_=YSB[:, ob, :])

        w8_p.release()
        xt8_p.release()
        w2a_p.release()
        aod_p.release()
        cons_p.release()
        vall_p.release()
        kall_p.release()
        qall_p.release()

    nc.compile()
    return nc


def host_prep(x, qkv_w, qkv_b, proj_w, proj_b, rel_pos_h, rel_pos_w):
    """full inputs -> list of 8 per-core in_maps"""
    import ml_dtypes
    F8NP = ml_dtypes.float8_e4m3
    BFNP = ml_dtypes.bfloat16

    x = np.asarray(x, np.float32)
    qkv_w = np.asarray(qkv_w, np.float32)
    qkv_b = np.asarray(qkv_b, np.float32)
    proj_w = np.asarray(proj_w, np.float32)
    proj_b = np.asarray(proj_b, np.float32)
    rel_pos_h = np.asarray(rel_pos_h, np.float32)
    rel_pos_w = np.asarray(rel_pos_w, np.float32)

    # qkv weights: transpose, 32x scale, fp8 hi/lo, [128, pair, slice, col]
    w32 = np.ascontiguousarray(qkv_w.T) * WS            # [768, 2304]
    w8hi = w32.astype(F8NP)
    w8lo = (w32 - w8hi.astype(np.float32)).astype(F8NP)
    w8hi = np.ascontiguousarray(
        w8hi.reshape(NPAIR, 2, 128, 3 * DIM).transpose(2, 0, 1, 3))
    w8lo = np.ascontiguousarray(
        w8lo.reshape(NPAIR, 2, 128, 3 * DIM).transpose(2, 0, 1, 3))

    qkvb18 = np.ascontiguousarray(qkv_b.reshape(18, 128).T)

    # proj: feature-major [768, 768]
    wprojT = np.ascontiguousarray(proj_w.T)
    projb6 = np.ascontiguousarray(proj_b.reshape(NC, 128).T)

    # rel-pos block-diagonal tables, fp8, new column placement
    idx = np.arange(H)
    Rh = rel_pos_h[idx[:, None] - idx[None, :] + (H - 1)]  # (32,32,64)
    Rw = rel_pos_w[idx[:, None] - idx[None, :] + (W - 1)]
    bd8 = np.zeros((2, 32, 128, 128), np.float32)
    for h in range(H):
        bd8[0, h, 0:64, 0:32] = Rh[h].T / SCALE     # even heads -> part 0:32
        bd8[0, h, 64:128, 64:96] = Rh[h].T / SCALE  # odd heads -> part 64:96
    for w in range(W):
        bd8[1, w, 0:64, 32:64] = Rw[w].T / SCALE
        bd8[1, w, 64:128, 96:128] = Rw[w].T / SCALE
    bd8 = np.ascontiguousarray(bd8.transpose(2, 0, 1, 3)).astype(F8NP)

    # indicator rows, replicated for both parities and all 6 pairs
    k = np.arange(N)
    kconst = np.zeros((64, N), np.float32)
    kconst[:32] = (k[None, :] // 32 == np.arange(32)[:, None])
    kconst[32:] = (k[None, :] % 32 == np.arange(32)[:, None])
    kc2 = np.concatenate([kconst, kconst], axis=0)      # [128, N]
    kc8rep = np.ascontiguousarray(
        np.broadcast_to(kc2[:, None, :], (128, NPR, N))).astype(F8NP)

    consd = np.zeros((128, 256), np.float32)
    consd[:, 0:128] = 1.0
    consd[:, 192:256] = 1.0
    vbrow32 = (qkv_b[2 * DIM:] * WS).reshape(1, DIM).astype(BFNP)

    shared = dict(w8hi=w8hi, w8lo=w8lo, qkvb18=qkvb18, bd8=bd8,
                  kc8rep=kc8rep, vbrow32=vbrow32, consd=consd,
                  wprojT=wprojT, projb6=projb6)
    in_maps = []
    for b in range(B):
        xT = np.ascontiguousarray(x[b].reshape(N, DIM).T)   # [768, 1024]
        x8hi = xT.astype(F8NP)
        x8lo = (xT - x8hi.astype(np.float32)).astype(F8NP)
        x8hi = np.ascontiguousarray(x8hi.reshape(NC, 128, N).transpose(1, 0, 2))
        x8lo = np.ascontiguousarray(x8lo.reshape(NC, 128, N).transpose(1, 0, 2))
        in_maps.append(dict(x8hi=x8hi, x8lo=x8lo, **shared))
    return in_maps


def get_nc():
    if "nc" not in _CACHE:
        _CACHE["nc"] = build_nc()
    return _CACHE["nc"]


def kernel(**inputs):
    nc = get_nc()
    in_maps = host_prep(**inputs)
    res = bass_utils.run_bass_kernel_spmd(nc, in_maps, core_ids=list(range(NCORES)))
    out = np.stack([np.asarray(r["y"]).T for r in res.results], axis=0)
    return np.ascontiguousarray(out).reshape(B, H, W, DIM).astype(np.float32)
